# revision 1
# baseline (speedup 1.0000x reference)
"""Trainium2 Bass kernel for nn_MAEEnhancedAttention (sparse attention).

Sharding: 8 cores = 2 batches x 4 head-groups (3 heads each). Each core
computes LN(q), LN(kv), the kv projection for its 3 heads, masked softmax
attention in transposed-score layout, and a partial dense projection.
The host sums the 4 per-batch partials and adds the bias corrections.

Per-core head offsets are baked out of the program by rotating the q
hidden-state / dense-weight columns on the host (np.roll), so the same
SPMD program serves all 8 cores.
"""

import functools
import sys

import numpy as np

try:
    import concourse.bass as bass  # noqa: F401
except Exception:  # pragma: no cover
    for p in ("/opt/trn_rl_repo", "/root/.axon_site/_ro/trn_rl_repo"):
        if p not in sys.path:
            sys.path.insert(0, p)

import ml_dtypes

import concourse.bass as bass
import concourse.mybir as mybir
import concourse.tile as tile
from concourse import bacc
from concourse.bass import ds, ts
from concourse.bass_utils import run_bass_kernel_spmd

BF16 = mybir.dt.bfloat16
FP32 = mybir.dt.float32
AF = mybir.ActivationFunctionType
ALU = mybir.AluOpType

B, S, SE, HID, H, D = 2, 2048, 2048, 768, 12, 64
L = SE + S            # 4096
NH = 3                # heads per core
DG = NH * D           # 192
P = 128
NCORES = 8
EPS = 1e-12
NT = S // P           # 16 s-tiles
NC_CHUNK = HID // P   # 6 contraction chunks
NLC = L // P          # 32 l-chunks
SB = 1024             # s block
NSB = S // SB         # 2
LB = 512              # l block for k projection
NLB = L // LB         # 8

NDVE_EXP = 0          # unmasked l-chunks per sweep using DVE bit-trick exp
EXP_A = float(184.6645544 * 0.125)   # 128*log2(e) * (1/sqrt(D))
EXP_B = float(16256.0 - 8.0)         # bf16 exponent bias + centering

TRACE = False
LAST_RESULTS = None   # BassKernelResults of the most recent run (for test.py)


def _body(tc, aps, general_gb):
    nc = tc.nc
    xq, xkv, enc_t, mask_t, wk_t, wv_t, wkb, wd_t, flagc, out = (
        aps["xq"], aps["xkv"], aps["enc_t"], aps["mask_t"], aps["wk_t"],
        aps["wv_t"], aps["wkb"], aps["wd_t"], aps["flagc"], aps["out"],
    )

    from contextlib import ExitStack
    with ExitStack() as ctx:
        # ---- long-lived pools -------------------------------------------
        wp = ctx.enter_context(tc.tile_pool(name="w", bufs=1))
        resq = ctx.enter_context(tc.tile_pool(name="lnq", bufs=NT))
        qdp = ctx.enter_context(tc.tile_pool(name="qd", bufs=1))
        kdp = ctx.enter_context(tc.tile_pool(name="kd", bufs=NLB))
        vp = ctx.enter_context(tc.tile_pool(name="vres", bufs=NLC))
        ekv_ctx = ctx.enter_context(__import__("contextlib").ExitStack())
        ekvp = ekv_ctx.enter_context(tc.tile_pool(name="ekv", bufs=2 * NC_CHUNK))

        # ---- weights / constants ----------------------------------------
        wk_sb = wp.tile([P, NC_CHUNK, DG], BF16, tag="wk")
        nc.sync.dma_start(wk_sb[:], wk_t.rearrange("(n p) d -> p n d", p=P))
        wv_sb = wp.tile([P, NC_CHUNK, DG], BF16, tag="wv")
        nc.sync.dma_start(wv_sb[:], wv_t.rearrange("(n p) d -> p n d", p=P))
        wd0 = wp.tile([P, HID], BF16, tag="wd0")
        nc.sync.dma_start(wd0[:], wd_t[0:P, :])
        wd1 = wp.tile([DG - P, HID], BF16, tag="wd1")
        nc.sync.dma_start(wd1[:], wd_t[P:DG, :])
        wkb0 = wp.tile([P, 1], FP32, tag="wkb0")
        nc.sync.dma_start(wkb0[:], wkb[0:P, :])
        wkb1 = wp.tile([DG - P, 1], FP32, tag="wkb1")
        nc.sync.dma_start(wkb1[:], wkb[P:DG, :])
        flag_sb = wp.tile([P, 1], FP32, tag="flag")
        nc.sync.dma_start(flag_sb[:], flagc[:, :])
        ident = wp.tile([P, P], BF16, tag="ident")
        from concourse.masks import make_identity
        make_identity(nc, ident[:])

        if general_gb:
            gbp = ekv_ctx.enter_context(tc.tile_pool(name="gb", bufs=1))
            bcs = {}
            for nm, src_ap in (("gq", aps["gq"]), ("bq", aps["bq"]),
                               ("gk", aps["gkv"]), ("bk", aps["bkv"])):
                row = gbp.tile([1, HID], FP32, tag=f"{nm}r", name=f"{nm}_r")
                nc.sync.dma_start(row[:], src_ap[:, :])
                rb = gbp.tile([1, HID], BF16, tag=f"{nm}rb", name=f"{nm}_rb")
                nc.vector.tensor_copy(rb[:], row[:])
                bct = gbp.tile([P, HID], BF16, tag=f"{nm}b", name=f"{nm}_bc")
                nc.gpsimd.partition_broadcast(bct[:], rb[:])
                bcs[nm] = bct
            gq_bc, bq_bc, gk_bc, bk_bc = (bcs["gq"], bcs["bq"],
                                          bcs["gk"], bcs["bk"])

        # resident tensors
        lnq = []            # 16 x [128, 768] bf16 (rotated col order)
        qd0 = qdp.tile([P, S], BF16, tag="qd0")      # q^T heads 0,1
        qd1 = qdp.tile([P, S], BF16, tag="qd1")      # q^T head 2 (rows 0:64)
        ekv_enc = []        # 6 x [128, 2048] bf16: enc^T chunks
        ekv_dec = []        # 6 x [128, 2048] bf16: LN(kv)^T chunks
        kd0_t = []          # 8 x [128, 512] bf16: k^T heads 0,1 per l-block
        kd1_t = []          # 8 x [64, 512] bf16: k^T head 2 per l-block
        v_tiles = []        # 32 x [128, 3, 66] bf16 (col 64 = ones)

        for c in range(NC_CHUNK):
            t = ekvp.tile([P, SE], BF16, tag="ekv")
            nc.sync.dma_start(t[:], enc_t[ts(c, P), :])
            ekv_enc.append(t)

        # ---- Phase A: LayerNorm + on-chip PE transposes -----------------
        for c in range(NC_CHUNK):
            t = ekvp.tile([P, S], BF16, tag="ekv", name=f"ekv_dec_{c}")
            ekv_dec.append(t)
        with tc.tile_pool(name="xin", bufs=8) as xin, \
             tc.tile_pool(name="stat", bufs=8) as stp, \
             tc.tile_pool(name="tp", bufs=2, space="PSUM") as tpp, \
             tc.tile_pool(name="astage", bufs=6) as astp, \
             tc.tile_pool(name="ltk", bufs=2) as ltkp:
            kb_buf = []
            qb_buf = []
            for i in range(NT):
                for which in ("kv", "q"):
                    src = xq if which == "q" else xkv
                    xt = xin.tile([P, HID], FP32, tag="xin")
                    nc.sync.dma_start(xt[:], src[ts(i, P), :])
                    st6 = stp.tile([P, 2, 6], FP32, tag="st6")
                    nc.vector.bn_stats(st6[:, 0, :], xt[:, 0:HID // 2])
                    nc.vector.bn_stats(st6[:, 1, :], xt[:, HID // 2:HID])
                    mv = stp.tile([P, 2], FP32, tag="mv")
                    nc.vector.bn_aggr(mv[:], st6[:])
                    sd = stp.tile([P, 1], FP32, tag="sd")
                    nc.vector.tensor_scalar_add(sd[:], mv[:, 1:2], EPS)
                    sq = stp.tile([P, 1], FP32, tag="sq")
                    nc.scalar.sqrt(sq[:], sd[:])
                    rs = stp.tile([P, 1], FP32, tag="rs")
                    nc.vector.reciprocal(rs[:], sq[:])
                    if which == "q":
                        lt = resq.tile([P, HID], FP32, tag="lnq")
                        nc.vector.tensor_scalar(
                            lt[:], xt[:], mv[:, 0:1], rs[:],
                            op0=ALU.subtract, op1=ALU.mult)
                        if general_gb:
                            nc.vector.tensor_mul(lt[:], lt[:], gq_bc[:])
                            nc.vector.tensor_add(lt[:], lt[:], bq_bc[:])
                        lnq.append(lt)
                        qb = astp.tile([P, 2 * P], BF16, tag="qb")
                        nc.vector.tensor_copy(qb[:], lt[:, 0:2 * P])
                        qb_buf.append(qb)
                        if len(qb_buf) == 4:
                            i0 = i - 3
                            for cc in range(2):
                                tp = tpp.tile([P, 4 * P], BF16, tag="tp",
                                              name=f"tq_{i0}_{cc}")
                                for j in range(4):
                                    nc.tensor.transpose(
                                        tp[:, ts(j, P)],
                                        qb_buf[j][:, ts(cc, P)], ident[:])
                                nc.scalar.copy(
                                    (qd0 if cc == 0 else qd1)
                                    [:, ds(i0 * P, 4 * P)], tp[:])
                            qb_buf = []
                    else:
                        if general_gb:
                            ltk = ltkp.tile([P, HID], FP32, tag="ltk")
                            nc.vector.tensor_scalar(
                                ltk[:], xt[:], mv[:, 0:1], rs[:],
                                op0=ALU.subtract, op1=ALU.mult)
                            nc.vector.tensor_mul(ltk[:], ltk[:], gk_bc[:])
                            kb = astp.tile([P, HID], BF16, tag="kb")
                            nc.vector.tensor_add(kb[:], ltk[:], bk_bc[:])
                        else:
                            kb = astp.tile([P, HID], BF16, tag="kb")
                            nc.gpsimd.tensor_scalar(
                                kb[:], xt[:], mv[:, 0:1], rs[:],
                                op0=ALU.subtract, op1=ALU.mult)
                        kb_buf.append(kb)
                        if len(kb_buf) == 4:
                            i0 = i - 3
                            for cc in range(NC_CHUNK):
                                tp = tpp.tile([P, 4 * P], BF16, tag="tp",
                                              name=f"tkv_{i0}_{cc}")
                                for j in range(4):
                                    nc.tensor.transpose(
                                        tp[:, ts(j, P)],
                                        kb_buf[j][:, ts(cc, P)], ident[:])
                                nc.scalar.copy(
                                    ekv_dec[cc][:, ds(i0 * P, 4 * P)], tp[:])
                            kb_buf = []
            qb_buf = []

        def ekv_rhs(c, lo, size):
            """rhs slice [128, size] of ekv^T chunk c covering l in [lo, lo+size)."""
            if lo < SE:
                assert lo + size <= SE
                return ekv_enc[c][:, ds(lo, size)]
            assert lo >= SE
            return ekv_dec[c][:, ds(lo - SE, size)]

        # ---- Phase B: kv projections ------------------------------------
        with tc.tile_pool(name="pk", bufs=3, space="PSUM") as pkp, \
             tc.tile_pool(name="pvps", bufs=3, space="PSUM") as pvp:
            for lb in range(NLB):
                for grp in range(2):
                    kbias = wkb0 if grp == 0 else wkb1
                    gp = P if grp == 0 else DG - P
                    pk = pkp.tile([P, LB], FP32, tag="pk")
                    for c in range(NC_CHUNK):
                        nc.tensor.matmul(
                            pk[0:gp, :],
                            lhsT=wk_sb[:, c, ds(grp * P, gp)],
                            rhs=ekv_rhs(c, lb * LB, LB),
                            start=(c == 0), stop=(c == NC_CHUNK - 1))
                    kt = kdp.tile([gp, LB], BF16, tag=f"kd{grp}",
                                  name=f"kd{grp}_{lb}")
                    nc.scalar.activation(kt[:], pk[0:gp, :], AF.Identity,
                                         bias=kbias[:], scale=1.0)
                    (kd0_t if grp == 0 else kd1_t).append(kt)
                for lt_i in range(4 * lb, 4 * lb + 4):
                    pv = pvp.tile([P, DG], FP32, tag="pv")
                    for c in range(NC_CHUNK):
                        nc.tensor.matmul(
                            pv[:],
                            lhsT=ekv_rhs(c, lt_i * P, P),
                            rhs=wv_sb[:, c, :],
                            start=(c == 0), stop=(c == NC_CHUNK - 1))
                    vt = vp.tile([P, NH, 66], BF16, tag="v",
                                 name=f"v_{lt_i}")
                    nc.scalar.copy(
                        vt[:, :, 0:D], pv[:].rearrange("p (h d) -> p h d", h=NH))
                    nc.gpsimd.memset(vt[:, :, D:D + 1], 1.0)
                    v_tiles.append(vt)

        ekv_ctx.close()

        # ---- mask: SBUF-resident, loaded once ---------------------------
        mask_res = []
        with tc.tile_pool(name="mask", bufs=NLC // 2) as mp:
            for i in range(NLC // 2):
                m_t = mp.tile([P, S], BF16, tag="m", name=f"mask_{i}")
                nc.sync.dma_start(m_t[:], mask_t[ts(i, P), :])
                mask_res.append(m_t)

            # ---- Phase C: attention + dense -----------------------------
            with tc.tile_pool(name="qk", bufs=2, space="PSUM") as qkp, \
                 tc.tile_pool(name="pvacc", bufs=1, space="PSUM") as pvap, \
                 tc.tile_pool(name="dps", bufs=2, space="PSUM") as dps, \
                 tc.tile_pool(name="pt", bufs=6) as ptp, \
                 tc.tile_pool(name="dn", bufs=2) as dnp, \
                 tc.tile_pool(name="att", bufs=3) as attp, \
                 tc.tile_pool(name="ob", bufs=3) as obp:
                HB = SB // 2  # psum-bank half of an s block
                for sb in range(NSB):
                    q_sl = [
                        qd0[0:D, ds(sb * SB, SB)],
                        qd0[D:2 * D, ds(sb * SB, SB)],
                        qd1[0:D, ds(sb * SB, SB)],
                    ]
                    att_a = attp.tile([P, SB], BF16, tag="atta")
                    att_b = attp.tile([D, SB], BF16, tag="attb")
                    att_sl = [att_a[0:D, :], att_a[D:2 * D, :], att_b[0:D, :]]
                    for h in range(NH):
                        pv_ps = pvap.tile([D + 1, SB], FP32, tag="pvacc",
                                          name=f"pvacc_{sb}_{h}")
                        for lc in range(NLC):
                            if h == 0:
                                ksl0 = kd0_t[lc // 4][0:D, ts(lc % 4, P)]
                            elif h == 1:
                                ksl0 = kd0_t[lc // 4][D:2 * D, ts(lc % 4, P)]
                            else:
                                ksl0 = kd1_t[lc // 4][0:D, ts(lc % 4, P)]
                            qk = qkp.tile([P, SB], FP32, tag="qk")
                            nc.tensor.matmul(qk[:, 0:HB], lhsT=ksl0,
                                             rhs=q_sl[h][:, 0:HB],
                                             start=True, stop=True)
                            nc.tensor.matmul(qk[:, HB:SB], lhsT=ksl0,
                                             rhs=q_sl[h][:, HB:SB],
                                             start=True, stop=True)
                            p_t = ptp.tile([P, SB], BF16, tag="p")
                            if lc < NDVE_EXP:
                                nc.vector.tensor_scalar(
                                    p_t[:].bitcast(mybir.dt.int16), qk[:],
                                    EXP_A, EXP_B,
                                    op0=ALU.mult, op1=ALU.add)
                            else:
                                nc.scalar.activation(
                                    p_t[:], qk[:], AF.Exp,
                                    scale=float(1.0 / np.sqrt(D)))
                            if lc >= NLC // 2:
                                nc.vector.tensor_mul(
                                    p_t[:], p_t[:],
                                    mask_res[lc - NLC // 2][:, ds(sb * SB, SB)])
                            nc.tensor.matmul(
                                pv_ps[:, 0:HB],
                                lhsT=v_tiles[lc][:, h, 0:D + 1],
                                rhs=p_t[:, 0:HB],
                                start=(lc == 0), stop=(lc == NLC - 1))
                            nc.tensor.matmul(
                                pv_ps[:, HB:SB],
                                lhsT=v_tiles[lc][:, h, 0:D + 1],
                                rhs=p_t[:, HB:SB],
                                start=(lc == 0), stop=(lc == NLC - 1))
                        # normalize by softmax denominator (row D of pv psum)
                        dn = dnp.tile([1, SB], FP32, tag="dn")
                        nc.vector.reciprocal(dn[:], pv_ps[D:D + 1, :])
                        bc = dnp.tile([D, SB], FP32, tag="bc")
                        nc.gpsimd.partition_broadcast(bc[:], dn[:])
                        nc.vector.tensor_mul(att_sl[h], pv_ps[0:D, :], bc[:])
                    # dense + residual for this s block
                    for st in range(SB // P):
                        gi = sb * (SB // P) + st
                        d1 = dps.tile([P, 512], FP32, tag="dp",
                                      name=f"d1_{sb}_{st}")
                        nc.tensor.matmul(d1[:], lhsT=att_a[:, ts(st, P)],
                                         rhs=wd0[:, 0:512],
                                         start=True, stop=False)
                        nc.tensor.matmul(d1[:], lhsT=att_b[:, ts(st, P)],
                                         rhs=wd1[:, 0:512],
                                         start=False, stop=True)
                        d2 = dps.tile([P, HID - 512], FP32, tag="dp",
                                      name=f"d2_{sb}_{st}")
                        nc.tensor.matmul(d2[:], lhsT=att_a[:, ts(st, P)],
                                         rhs=wd0[:, 512:HID],
                                         start=True, stop=False)
                        nc.tensor.matmul(d2[:], lhsT=att_b[:, ts(st, P)],
                                         rhs=wd1[:, 512:HID],
                                         start=False, stop=True)
                        ob = obp.tile([P, HID], FP32, tag="ob")
                        nc.vector.scalar_tensor_tensor(
                            ob[:, 0:512], lnq[gi][:, 0:512], flag_sb[:], d1[:],
                            op0=ALU.mult, op1=ALU.add)
                        nc.vector.scalar_tensor_tensor(
                            ob[:, 512:HID], lnq[gi][:, 512:HID], flag_sb[:],
                            d2[:], op0=ALU.mult, op1=ALU.add)
                        nc.sync.dma_start(out[ts(gi, P), :], ob[:])


@functools.lru_cache(maxsize=2)
def _build(general_gb):
    nc = bacc.Bacc("TRN2", target_bir_lowering=False, debug=False)
    aps = {
        "xq": nc.dram_tensor("xq", [S, HID], FP32, kind="ExternalInput").ap(),
        "xkv": nc.dram_tensor("xkv", [S, HID], FP32, kind="ExternalInput").ap(),
        "enc_t": nc.dram_tensor("enc_t", [HID, SE], BF16, kind="ExternalInput").ap(),
        "mask_t": nc.dram_tensor("mask_t", [S, S], BF16, kind="ExternalInput").ap(),
        "wk_t": nc.dram_tensor("wk_t", [HID, DG], BF16, kind="ExternalInput").ap(),
        "wv_t": nc.dram_tensor("wv_t", [HID, DG], BF16, kind="ExternalInput").ap(),
        "wkb": nc.dram_tensor("wkb", [DG, 1], FP32, kind="ExternalInput").ap(),
        "wd_t": nc.dram_tensor("wd_t", [DG, HID], BF16, kind="ExternalInput").ap(),
        "flagc": nc.dram_tensor("flagc", [P, 1], FP32, kind="ExternalInput").ap(),
        "out": nc.dram_tensor("out", [S, HID], FP32, kind="ExternalOutput").ap(),
    }
    if general_gb:
        for n in ("gq", "bq", "gkv", "bkv"):
            aps[n] = nc.dram_tensor(n, [1, HID], FP32, kind="ExternalInput").ap()
    with tile.TileContext(nc) as tc:
        _body(tc, aps, general_gb)
    nc.compile()
    return nc


def _bf16(a):
    return np.ascontiguousarray(np.asarray(a, np.float32)).astype(ml_dtypes.bfloat16)


def make_in_maps(query_hidden_states, key_value_hidden_states, encoder_output,
                 attention_mask, decoding_mask, Wkv_w, Wkv_b, dense_w,
                 norm_g, norm_b, general_gb):
    eye = np.eye(S, dtype=bool)
    in_maps = []
    for c in range(NCORES):
        b, g = divmod(c, 4)
        h0c = g * DG
        m = (np.asarray(attention_mask[b], bool)[None, :]
             & np.asarray(decoding_mask[b], bool) & ~eye)
        im = {
            "xq": np.ascontiguousarray(
                np.roll(np.asarray(query_hidden_states[b], np.float32), -h0c, axis=1)),
            "xkv": np.ascontiguousarray(
                np.asarray(key_value_hidden_states[b], np.float32)),
            "enc_t": _bf16(np.asarray(encoder_output[b], np.float32).T),
            "mask_t": np.ascontiguousarray(m.T).astype(ml_dtypes.bfloat16),
            "wk_t": _bf16(np.asarray(Wkv_w, np.float32)[h0c:h0c + DG, :].T),
            "wv_t": _bf16(np.asarray(Wkv_w, np.float32)[HID + h0c:HID + h0c + DG, :].T),
            "wkb": np.ascontiguousarray(
                np.asarray(Wkv_b, np.float32)[h0c:h0c + DG].reshape(DG, 1)),
            "wd_t": _bf16(np.roll(
                np.asarray(dense_w, np.float32)[:, h0c:h0c + DG].T, -h0c, axis=1)),
            "flagc": np.full((P, 1), 1.0 if g == 0 else 0.0, np.float32),
        }
        if general_gb:
            im["gq"] = np.ascontiguousarray(
                np.roll(np.asarray(norm_g, np.float32), -h0c)[None, :])
            im["bq"] = np.ascontiguousarray(
                np.roll(np.asarray(norm_b, np.float32), -h0c)[None, :])
            im["gkv"] = np.ascontiguousarray(np.asarray(norm_g, np.float32)[None, :])
            im["bkv"] = np.ascontiguousarray(np.asarray(norm_b, np.float32)[None, :])
        in_maps.append(im)
    return in_maps


def kernel(query_hidden_states, key_value_hidden_states, encoder_output,
           attention_mask, decoding_mask, Wq_w, Wq_b, Wkv_w, Wkv_b,
           dense_w, dense_b, norm_g, norm_b):
    # Wq output is discarded by the reference; Wq_w/Wq_b intentionally unused.
    global LAST_RESULTS
    norm_g = np.asarray(norm_g, np.float32)
    norm_b = np.asarray(norm_b, np.float32)
    general_gb = not (np.all(norm_g == 1.0) and np.all(norm_b == 0.0))
    nc = _build(general_gb)
    in_maps = make_in_maps(
        query_hidden_states, key_value_hidden_states, encoder_output,
        attention_mask, decoding_mask, Wkv_w, Wkv_b, dense_w,
        norm_g, norm_b, general_gb)
    try:
        res = run_bass_kernel_spmd(nc, in_maps, core_ids=list(range(NCORES)),
                                   trace=TRACE)
    except ModuleNotFoundError:
        res = run_bass_kernel_spmd(nc, in_maps, core_ids=list(range(NCORES)),
                                   trace=False)
    LAST_RESULTS = res
    outs = [r["out"] for r in res.results]
    dense_b = np.asarray(dense_b, np.float32)
    corr = dense_b + np.asarray(dense_w, np.float32) @ np.asarray(
        Wkv_b, np.float32)[HID:]
    full = np.zeros((B, S, HID), np.float32)
    for c in range(NCORES):
        b, g = divmod(c, 4)
        full[b] += np.roll(np.asarray(outs[c], np.float32), g * DG, axis=1)
    full += corr[None, None, :]
    return full


def bench_hw(iters=5, **inputs):
    """Time warm executions with device-resident inputs (excludes host prep).

    Returns (best_seconds, results_list_for_core_outputs).
    """
    import time

    import jax
    from jax.experimental.shard_map import shard_map
    from jax.sharding import Mesh, PartitionSpec

    from concourse import bass2jax
    from concourse.bass2jax import _bass_exec_p, install_neuronx_cc_hook
    import concourse.mybir as mybir_

    norm_g = np.asarray(inputs["norm_g"], np.float32)
    norm_b = np.asarray(inputs["norm_b"], np.float32)
    general_gb = not (np.all(norm_g == 1.0) and np.all(norm_b == 0.0))
    nc = _build(general_gb)
    in_maps = make_in_maps(
        inputs["query_hidden_states"], inputs["key_value_hidden_states"],
        inputs["encoder_output"], inputs["attention_mask"],
        inputs["decoding_mask"], inputs["Wkv_w"], inputs["Wkv_b"],
        inputs["dense_w"], norm_g, norm_b, general_gb)

    install_neuronx_cc_hook()
    n_cores = NCORES
    partition_name = (nc.partition_id_tensor.name
                      if nc.partition_id_tensor else None)
    in_names, out_names, out_avals, zero_outs = [], [], [], []
    for alloc in nc.m.functions[0].allocations:
        if not isinstance(alloc, mybir_.MemoryLocationSet):
            continue
        name = alloc.memorylocations[0].name
        if alloc.kind == "ExternalInput":
            if name != partition_name:
                in_names.append(name)
        elif alloc.kind == "ExternalOutput":
            out_names.append(name)
            shape = tuple(alloc.tensor_shape)
            dtype = mybir_.dt.np(alloc.dtype)
            out_avals.append(jax.core.ShapedArray(shape, dtype))
            zero_outs.append(np.zeros(shape, dtype))
    n_params = len(in_names)
    all_names = in_names + out_names
    if partition_name is not None:
        all_names.append(partition_name)

    def _body(*args):
        operands = list(args)
        if partition_name is not None:
            operands.append(bass2jax.partition_id_tensor())
        outs = _bass_exec_p.bind(
            *operands, out_avals=tuple(out_avals), in_names=tuple(all_names),
            out_names=tuple(out_names), lowering_input_output_aliases=(),
            sim_require_finite=True, sim_require_nnan=True, nc=nc)
        return tuple(outs)

    devices = jax.devices()[:n_cores]
    mesh = Mesh(np.asarray(devices), ("core",))
    n_outs = len(out_names)
    sharded = jax.jit(
        shard_map(_body, mesh=mesh,
                  in_specs=(PartitionSpec("core"),) * (n_params + n_outs),
                  out_specs=(PartitionSpec("core"),) * n_outs,
                  check_rep=False),
        donate_argnums=tuple(range(n_params, n_params + n_outs)),
        keep_unused=True)
    concat_in = [
        np.concatenate([np.asarray(in_maps[c][nm]) for c in range(n_cores)], 0)
        for nm in in_names]
    dev_in = [jax.device_put(a) for a in concat_in]
    concat_zeros = [np.zeros((n_cores * z.shape[0], *z.shape[1:]), z.dtype)
                    for z in zero_outs]

    times = []
    outs = None
    for _ in range(iters):
        zs = [jax.device_put(z) for z in concat_zeros]
        jax.block_until_ready(zs)
        jax.block_until_ready(dev_in)
        t0 = time.perf_counter()
        outs = sharded(*dev_in, *zs)
        jax.block_until_ready(outs)
        times.append(time.perf_counter() - t0)
    # slope estimate: issue NB calls back-to-back, block once at the end.
    # amortizes the axon-tunnel round-trip; (tN - t1)/(NB-1) ~ per-exec.
    NB = 8
    zsets = [[jax.device_put(z) for z in concat_zeros] for _ in range(NB)]
    jax.block_until_ready(zsets)
    t0 = time.perf_counter()
    outs1 = sharded(*dev_in, *zsets[0])
    jax.block_until_ready(outs1)
    t1 = time.perf_counter() - t0
    t0 = time.perf_counter()
    many = [sharded(*dev_in, *zsets[i]) for i in range(1, NB)]
    jax.block_until_ready(many)
    tn = time.perf_counter() - t0
    slope = tn / (NB - 1)
    print(f"bench slope: 1-call {t1 * 1e3:.2f} ms, "
          f"{NB - 1} calls {tn * 1e3:.2f} ms -> {slope * 1e3:.3f} ms/exec")
    results = [
        {nm: np.asarray(outs[i]).reshape(n_cores, *out_avals[i].shape)[c]
         for i, nm in enumerate(out_names)}
        for c in range(n_cores)]
    return min(min(times), slope), times + [slope], results



# revision 4
# speedup vs baseline: 1.1570x; 1.1570x over previous
"""Trainium2 Bass kernel for nn_MAEEnhancedAttention (sparse attention).

Sharding: 8 cores = 2 batches x 4 s-slices (512 query rows each). Each core
computes LN(q) for its rows, LN(kv) for the full batch, the full 12-head
k/v projection, masked softmax attention in transposed-score layout, the
dense projection and residual for its disjoint row slice. No host-side
reduction: outputs are disjoint [512, 768] slices.

The axon tunnel re-ships operand bytes on every execution, so all large
inputs are bf16, the mask ships as uint8 (converted once on device), and
the output is bf16 — this is what the exec-time metric is dominated by.

k-bias is dropped (softmax is invariant to a per-row constant shift);
v-bias and dense bias are folded into a host-side per-column constant.
"""

import functools
import sys

import numpy as np

try:
    import concourse.bass as bass  # noqa: F401
except Exception:  # pragma: no cover
    for p in ("/opt/trn_rl_repo", "/root/.axon_site/_ro/trn_rl_repo"):
        if p not in sys.path:
            sys.path.insert(0, p)

import ml_dtypes

import concourse.bass as bass
import concourse.mybir as mybir
import concourse.tile as tile
from concourse import bacc
from concourse.bass import ds, ts
from concourse.bass_utils import run_bass_kernel_spmd

BF16 = mybir.dt.bfloat16
FP32 = mybir.dt.float32
U8 = mybir.dt.uint8
AF = mybir.ActivationFunctionType
ALU = mybir.AluOpType

B, S, SE, HID, H, D = 2, 2048, 2048, 768, 12, 64
L = SE + S            # 4096
SS = S // 4           # 512 query rows per core
P = 128
NCORES = 8
EPS = 1e-12
NC_CHUNK = HID // P   # 6 contraction chunks
NLC = L // P          # 32 l-chunks
LB = 512              # l block
NLB = L // LB         # 8
NPAIR = H // 2        # 6 head pairs

TRACE = False
LAST_RESULTS = None   # BassKernelResults of the most recent run (for test.py)


def _body(tc, aps, general_gb):
    nc = tc.nc
    xq, xkv, enc3, mask_u8, wk3, wv3, wd3, out = (
        aps["xq"], aps["xkv"], aps["enc3"], aps["mask_u8"], aps["wk3"],
        aps["wv3"], aps["wd3"], aps["out"],
    )

    from contextlib import ExitStack
    with ExitStack() as ctx:
        # ---- long-lived pools -------------------------------------------
        wp = ctx.enter_context(tc.tile_pool(name="w", bufs=1))
        lnqp = ctx.enter_context(tc.tile_pool(name="lnq", bufs=4))
        qdp = ctx.enter_context(tc.tile_pool(name="qd", bufs=NPAIR))
        kdp = ctx.enter_context(tc.tile_pool(name="kd", bufs=NPAIR))
        vp = ctx.enter_context(tc.tile_pool(name="vres", bufs=NLC))
        wkv_ctx = ctx.enter_context(__import__("contextlib").ExitStack())
        wkvp = wkv_ctx.enter_context(tc.tile_pool(name="wkv", bufs=2))

        # ---- weights / constants ----------------------------------------
        wk_sb = wkvp.tile([P, NC_CHUNK, HID], BF16, tag="wk")
        nc.sync.dma_start(wk_sb[:], wk3[:, :, :])
        wv_sb = wkvp.tile([P, NC_CHUNK, HID], BF16, tag="wv")
        nc.sync.dma_start(wv_sb[:], wv3[:, :, :])
        wd_sb = wp.tile([P, NC_CHUNK, HID], BF16, tag="wd")
        nc.sync.dma_start(wd_sb[:], wd3[:, :, :])
        ident = wp.tile([P, P], BF16, tag="ident")
        from concourse.masks import make_identity
        make_identity(nc, ident[:])

        if general_gb:
            gbp = ctx.enter_context(tc.tile_pool(name="gb", bufs=1))
            bcs = {}
            for nm in ("g", "b"):
                row = gbp.tile([1, HID], FP32, tag=f"{nm}r", name=f"{nm}_r")
                nc.sync.dma_start(row[:], aps[nm + "_r"][:, :])
                bct = gbp.tile([P, HID], FP32, tag=f"{nm}b", name=f"{nm}_bc")
                nc.gpsimd.partition_broadcast(bct[:], row[:])
                bcs[nm] = bct
            g_bc, b_bc = bcs["g"], bcs["b"]

        # resident tensors
        lnq = []            # 4 x [128, 768] f32 (residual for our rows)
        qd = []             # 6 x [128, 512] bf16: q^T head pairs
        kd = []             # 6 x [128, 4096] bf16: k^T head pairs
        v_tiles = []        # 32 x [128, 12, 66] bf16 (col 64 = ones)
        for j in range(NPAIR):
            kd.append(kdp.tile([P, L], BF16, tag="kd", name=f"kd_{j}"))
        for lt_i in range(NLC):
            v_tiles.append(vp.tile([P, H, 66], BF16, tag="v",
                                   name=f"v_{lt_i}"))

        def ln_tile(pool_st, xt, out_tile, out_slice=None):
            """LayerNorm stats for one [128, 768] tile; returns (mean, rstd)."""
            st6 = pool_st.tile([P, 2, 6], FP32, tag="st6")
            nc.vector.bn_stats(st6[:, 0, :], xt[:, 0:HID // 2])
            nc.vector.bn_stats(st6[:, 1, :], xt[:, HID // 2:HID])
            mv = pool_st.tile([P, 2], FP32, tag="mv")
            nc.vector.bn_aggr(mv[:], st6[:])
            sd = pool_st.tile([P, 1], FP32, tag="sd")
            nc.vector.tensor_scalar_add(sd[:], mv[:, 1:2], EPS)
            sq = pool_st.tile([P, 1], FP32, tag="sq")
            nc.scalar.sqrt(sq[:], sd[:])
            rs = pool_st.tile([P, 1], FP32, tag="rs")
            nc.vector.reciprocal(rs[:], sq[:])
            return mv, rs

        # ---- Phase A: LN(q) + q^T ---------------------------------------
        with tc.tile_pool(name="xin", bufs=4) as xin, \
             tc.tile_pool(name="stat", bufs=8) as stp, \
             tc.tile_pool(name="tpq", bufs=2, space="PSUM") as tpq, \
             tc.tile_pool(name="qstage", bufs=4) as qst:
            qb_buf = []
            for i in range(SS // P):
                xt = xin.tile([P, HID], BF16, tag="xin")
                nc.sync.dma_start(xt[:], xq[ts(i, P), :])
                mv, rs = ln_tile(stp, xt, None)
                lt = lnqp.tile([P, HID], FP32, tag="lnq", name=f"lnq_{i}")
                nc.vector.tensor_scalar(
                    lt[:], xt[:], mv[:, 0:1], rs[:],
                    op0=ALU.subtract, op1=ALU.mult)
                if general_gb:
                    nc.vector.tensor_mul(lt[:], lt[:], g_bc[:])
                    nc.vector.tensor_add(lt[:], lt[:], b_bc[:])
                lnq.append(lt)
                qb = qst.tile([P, HID], BF16, tag="qb")
                nc.vector.tensor_copy(qb[:], lt[:])
                qb_buf.append(qb)
            for cc in range(NC_CHUNK):
                tp = tpq.tile([P, SS], BF16, tag="tpq", name=f"tq_{cc}")
                for j in range(SS // P):
                    nc.tensor.transpose(
                        tp[:, ts(j, P)], qb_buf[j][:, ts(cc, P)], ident[:])
                qt = qdp.tile([P, SS], BF16, tag="qd", name=f"qd_{cc}")
                nc.scalar.copy(qt[:], tp[:])
                qd.append(qt)

        # ---- Phase B: streamed ekv^T + k/v projections ------------------
        with tc.tile_pool(name="kvin", bufs=8) as kvin, \
             tc.tile_pool(name="statb", bufs=8) as stb, \
             tc.tile_pool(name="tpk", bufs=2, space="PSUM") as tpk, \
             tc.tile_pool(name="ebp", bufs=2) as ebp, \
             tc.tile_pool(name="kstage", bufs=5) as kst, \
             tc.tile_pool(name="pk", bufs=2, space="PSUM") as pkp, \
             tc.tile_pool(name="pv", bufs=2, space="PSUM") as pvp:
            for lb in range(NLB):
                # -- obtain ekv^T block eb[c]: [128, 512] for this l-block
                if lb < SE // LB:
                    eb_t = ebp.tile([P, NC_CHUNK, LB], BF16, tag="eb",
                                    name=f"eb_{lb}")
                    nc.sync.dma_start(eb_t[:], enc3[:, :, ts(lb, LB)])
                    eb = [eb_t[:, c, :] for c in range(NC_CHUNK)]
                else:
                    kb_buf = []
                    for jj in range(LB // P):
                        i = (lb - SE // LB) * (LB // P) + jj
                        xt = kvin.tile([P, HID], BF16, tag="kvin")
                        nc.sync.dma_start(xt[:], xkv[ts(i, P), :])
                        mv, rs = ln_tile(stb, xt, None)
                        if general_gb:
                            ltk = kst.tile([P, HID], FP32, tag="ltk")
                            nc.vector.tensor_scalar(
                                ltk[:], xt[:], mv[:, 0:1], rs[:],
                                op0=ALU.subtract, op1=ALU.mult)
                            nc.vector.tensor_mul(ltk[:], ltk[:], g_bc[:])
                            kb = kst.tile([P, HID], BF16, tag="kb")
                            nc.vector.tensor_add(kb[:], ltk[:], b_bc[:])
                        else:
                            kb = kst.tile([P, HID], BF16, tag="kb")
                            nc.gpsimd.tensor_scalar(
                                kb[:], xt[:], mv[:, 0:1], rs[:],
                                op0=ALU.subtract, op1=ALU.mult)
                        kb_buf.append(kb)
                    eb_t = ebp.tile([P, NC_CHUNK, LB], BF16, tag="eb",
                                    name=f"eb_{lb}")
                    for cc in range(NC_CHUNK):
                        tp = tpk.tile([P, LB], BF16, tag="tpk",
                                      name=f"tkv_{lb}_{cc}")
                        for j in range(LB // P):
                            nc.tensor.transpose(
                                tp[:, ts(j, P)], kb_buf[j][:, ts(cc, P)],
                                ident[:])
                        nc.scalar.copy(eb_t[:, cc, :], tp[:])
                    eb = [eb_t[:, c, :] for c in range(NC_CHUNK)]
                # -- k^T for this l-block: 6 head-pair groups
                for g in range(NPAIR):
                    pk = pkp.tile([P, LB], FP32, tag="pk")
                    for c in range(NC_CHUNK):
                        nc.tensor.matmul(
                            pk[:], lhsT=wk_sb[:, c, ts(g, P)], rhs=eb[c],
                            start=(c == 0), stop=(c == NC_CHUNK - 1))
                    nc.scalar.copy(kd[g][:, ts(lb, LB)], pk[:])
                # -- v for the 4 l-tiles of this block
                for jj in range(LB // P):
                    lt_i = lb * (LB // P) + jj
                    pv = pvp.tile([P, HID], FP32, tag="pv")
                    for c in range(NC_CHUNK):
                        nc.tensor.matmul(
                            pv[:, 0:512], lhsT=eb[c][:, ts(jj, P)],
                            rhs=wv_sb[:, c, 0:512],
                            start=(c == 0), stop=(c == NC_CHUNK - 1))
                    for c in range(NC_CHUNK):
                        nc.tensor.matmul(
                            pv[:, 512:HID], lhsT=eb[c][:, ts(jj, P)],
                            rhs=wv_sb[:, c, 512:HID],
                            start=(c == 0), stop=(c == NC_CHUNK - 1))
                    vt = v_tiles[lt_i]
                    nc.scalar.copy(
                        vt[:, 0:8, 0:D],
                        pv[:, 0:512].rearrange("p (h d) -> p h d", h=8))
                    nc.scalar.copy(
                        vt[:, 8:H, 0:D],
                        pv[:, 512:HID].rearrange("p (h d) -> p h d", h=4))
                    nc.gpsimd.memset(vt[:, :, D:D + 1], 1.0)

        wkv_ctx.close()

        # ---- mask: uint8 -> bf16, SBUF-resident -------------------------
        mask_res = []
        with tc.tile_pool(name="mu8", bufs=4) as mup, \
             tc.tile_pool(name="mask", bufs=NLC // 2) as mp:
            for i in range(NLC // 2):
                mu = mup.tile([P, SS], U8, tag="mu8")
                nc.sync.dma_start(mu[:], mask_u8[ts(i, P), :])
                m_t = mp.tile([P, SS], BF16, tag="m", name=f"mask_{i}")
                nc.vector.tensor_copy(m_t[:], mu[:])
                mask_res.append(m_t)

            # ---- Phase C: attention + dense -----------------------------
            with tc.tile_pool(name="qk", bufs=2, space="PSUM") as qkp, \
                 tc.tile_pool(name="pvacc", bufs=2, space="PSUM") as pvap, \
                 tc.tile_pool(name="dps", bufs=2, space="PSUM") as dps, \
                 tc.tile_pool(name="pt", bufs=6) as ptp, \
                 tc.tile_pool(name="dn", bufs=4) as dnp, \
                 tc.tile_pool(name="att", bufs=NPAIR) as attp, \
                 tc.tile_pool(name="ob", bufs=3) as obp:
                att = []
                for j in range(NPAIR):
                    pva = pvap.tile([D + 1, SS], FP32, tag="pvacc",
                                    name=f"pva_{j}")
                    pvb = pvap.tile([D + 1, SS], FP32, tag="pvacc",
                                    name=f"pvb_{j}")
                    for lc in range(NLC):
                        qk = qkp.tile([P, 2 * SS], FP32, tag="qk")
                        nc.tensor.matmul(qk[:, 0:SS],
                                         lhsT=kd[j][0:D, ts(lc, P)],
                                         rhs=qd[j][0:D, :],
                                         start=True, stop=True)
                        nc.tensor.matmul(qk[:, SS:2 * SS],
                                         lhsT=kd[j][D:2 * D, ts(lc, P)],
                                         rhs=qd[j][D:2 * D, :],
                                         start=True, stop=True)
                        p_t = ptp.tile([P, 2 * SS], BF16, tag="p")
                        nc.scalar.activation(
                            p_t[:], qk[:], AF.Exp,
                            scale=float(1.0 / np.sqrt(D)))
                        if lc >= NLC // 2:
                            m_t = mask_res[lc - NLC // 2]
                            nc.vector.tensor_mul(
                                p_t[:, 0:SS], p_t[:, 0:SS], m_t[:])
                            nc.vector.tensor_mul(
                                p_t[:, SS:2 * SS], p_t[:, SS:2 * SS], m_t[:])
                        nc.tensor.matmul(
                            pva[:], lhsT=v_tiles[lc][:, 2 * j, 0:D + 1],
                            rhs=p_t[:, 0:SS],
                            start=(lc == 0), stop=(lc == NLC - 1))
                        nc.tensor.matmul(
                            pvb[:], lhsT=v_tiles[lc][:, 2 * j + 1, 0:D + 1],
                            rhs=p_t[:, SS:2 * SS],
                            start=(lc == 0), stop=(lc == NLC - 1))
                    at = attp.tile([P, SS], BF16, tag="att", name=f"att_{j}")
                    for half, pvx in ((0, pva), (1, pvb)):
                        dn = dnp.tile([1, SS], FP32, tag="dn")
                        nc.vector.reciprocal(dn[:], pvx[D:D + 1, :])
                        bc = dnp.tile([D, SS], FP32, tag="bc")
                        nc.gpsimd.partition_broadcast(bc[:], dn[:])
                        nc.vector.tensor_mul(
                            at[ds(half * D, D), :], pvx[0:D, :], bc[:])
                    att.append(at)
                # dense + residual
                for st in range(SS // P):
                    d1 = dps.tile([P, 512], FP32, tag="dp",
                                  name=f"d1_{st}")
                    for j in range(NPAIR):
                        nc.tensor.matmul(d1[:], lhsT=att[j][:, ts(st, P)],
                                         rhs=wd_sb[:, j, 0:512],
                                         start=(j == 0), stop=(j == NPAIR - 1))
                    d2 = dps.tile([P, HID - 512], FP32, tag="dp",
                                  name=f"d2_{st}")
                    for j in range(NPAIR):
                        nc.tensor.matmul(d2[:], lhsT=att[j][:, ts(st, P)],
                                         rhs=wd_sb[:, j, 512:HID],
                                         start=(j == 0), stop=(j == NPAIR - 1))
                    ob = obp.tile([P, HID], BF16, tag="ob")
                    nc.vector.tensor_add(ob[:, 0:512], lnq[st][:, 0:512], d1[:])
                    nc.vector.tensor_add(ob[:, 512:HID], lnq[st][:, 512:HID],
                                         d2[:])
                    nc.sync.dma_start(out[ts(st, P), :], ob[:])


@functools.lru_cache(maxsize=2)
def _build(general_gb):
    nc = bacc.Bacc("TRN2", target_bir_lowering=False, debug=False)
    aps = {
        "xq": nc.dram_tensor("xq", [SS, HID], BF16, kind="ExternalInput").ap(),
        "xkv": nc.dram_tensor("xkv", [S, HID], BF16, kind="ExternalInput").ap(),
        "enc3": nc.dram_tensor("enc3", [P, NC_CHUNK, SE], BF16,
                               kind="ExternalInput").ap(),
        "mask_u8": nc.dram_tensor("mask_u8", [S, SS], U8,
                                  kind="ExternalInput").ap(),
        "wk3": nc.dram_tensor("wk3", [P, NC_CHUNK, HID], BF16,
                              kind="ExternalInput").ap(),
        "wv3": nc.dram_tensor("wv3", [P, NC_CHUNK, HID], BF16,
                              kind="ExternalInput").ap(),
        "wd3": nc.dram_tensor("wd3", [P, NC_CHUNK, HID], BF16,
                              kind="ExternalInput").ap(),
        "out": nc.dram_tensor("out", [SS, HID], BF16, kind="ExternalOutput").ap(),
    }
    if general_gb:
        for n in ("g_r", "b_r"):
            aps[n] = nc.dram_tensor(n, [1, HID], FP32, kind="ExternalInput").ap()
    with tile.TileContext(nc) as tc:
        _body(tc, aps, general_gb)
    nc.compile()
    return nc


def _bf16(a):
    return np.ascontiguousarray(np.asarray(a, np.float32)).astype(ml_dtypes.bfloat16)


def _w3(w_t):
    """[768, 768] (already transposed) -> [128, 6, 768] partition-major."""
    return np.ascontiguousarray(
        _bf16(w_t).reshape(NC_CHUNK, P, HID).transpose(1, 0, 2))


def make_in_maps(query_hidden_states, key_value_hidden_states, encoder_output,
                 attention_mask, decoding_mask, Wkv_w, dense_w,
                 norm_g, norm_b, general_gb):
    eye = np.eye(S, dtype=bool)
    Wkv = np.asarray(Wkv_w, np.float32)
    wk3 = _w3(Wkv[0:HID, :].T)
    wv3 = _w3(Wkv[HID:2 * HID, :].T)
    wd3 = _w3(np.asarray(dense_w, np.float32).T)
    per_batch = []
    for b in range(B):
        xkv = _bf16(key_value_hidden_states[b])
        enc3 = np.ascontiguousarray(
            _bf16(np.asarray(encoder_output[b], np.float32).T)
            .reshape(NC_CHUNK, P, SE).transpose(1, 0, 2))
        m = (np.asarray(attention_mask[b], bool)[None, :]
             & np.asarray(decoding_mask[b], bool) & ~eye)
        per_batch.append((xkv, enc3, m))
    in_maps = []
    for c in range(NCORES):
        b, sl = divmod(c, 4)
        xkv, enc3, m = per_batch[b]
        r0 = sl * SS
        im = {
            "xq": _bf16(np.asarray(query_hidden_states[b], np.float32)
                        [r0:r0 + SS]),
            "xkv": xkv,
            "enc3": enc3,
            "mask_u8": np.ascontiguousarray(
                m[r0:r0 + SS, :].T).astype(np.uint8),
            "wk3": wk3,
            "wv3": wv3,
            "wd3": wd3,
        }
        if general_gb:
            im["g_r"] = np.ascontiguousarray(np.asarray(norm_g, np.float32)[None, :])
            im["b_r"] = np.ascontiguousarray(np.asarray(norm_b, np.float32)[None, :])
        in_maps.append(im)
    return in_maps


def kernel(query_hidden_states, key_value_hidden_states, encoder_output,
           attention_mask, decoding_mask, Wq_w, Wq_b, Wkv_w, Wkv_b,
           dense_w, dense_b, norm_g, norm_b):
    # Wq output is discarded by the reference; Wq_w/Wq_b intentionally unused.
    global LAST_RESULTS
    norm_g = np.asarray(norm_g, np.float32)
    norm_b = np.asarray(norm_b, np.float32)
    general_gb = not (np.all(norm_g == 1.0) and np.all(norm_b == 0.0))
    nc = _build(general_gb)
    in_maps = make_in_maps(
        query_hidden_states, key_value_hidden_states, encoder_output,
        attention_mask, decoding_mask, Wkv_w, dense_w,
        norm_g, norm_b, general_gb)
    try:
        res = run_bass_kernel_spmd(nc, in_maps, core_ids=list(range(NCORES)),
                                   trace=TRACE)
    except ModuleNotFoundError:
        res = run_bass_kernel_spmd(nc, in_maps, core_ids=list(range(NCORES)),
                                   trace=False)
    LAST_RESULTS = res
    dense_b = np.asarray(dense_b, np.float32)
    corr = dense_b + np.asarray(dense_w, np.float32) @ np.asarray(
        Wkv_b, np.float32)[HID:]
    full = np.empty((B, S, HID), np.float32)
    for c in range(NCORES):
        b, sl = divmod(c, 4)
        full[b, sl * SS:(sl + 1) * SS] = np.asarray(
            res.results[c]["out"], np.float32)
    full += corr[None, None, :]
    return full


def bench_hw(iters=5, **inputs):
    """Time warm executions with device-resident inputs (excludes host prep).

    Returns (best_seconds, times_list, results_list_for_core_outputs).
    """
    import time

    import jax
    from jax.experimental.shard_map import shard_map
    from jax.sharding import Mesh, PartitionSpec

    from concourse import bass2jax
    from concourse.bass2jax import _bass_exec_p, install_neuronx_cc_hook
    import concourse.mybir as mybir_

    norm_g = np.asarray(inputs["norm_g"], np.float32)
    norm_b = np.asarray(inputs["norm_b"], np.float32)
    general_gb = not (np.all(norm_g == 1.0) and np.all(norm_b == 0.0))
    nc = _build(general_gb)
    in_maps = make_in_maps(
        inputs["query_hidden_states"], inputs["key_value_hidden_states"],
        inputs["encoder_output"], inputs["attention_mask"],
        inputs["decoding_mask"], inputs["Wkv_w"],
        inputs["dense_w"], norm_g, norm_b, general_gb)

    install_neuronx_cc_hook()
    n_cores = NCORES
    partition_name = (nc.partition_id_tensor.name
                      if nc.partition_id_tensor else None)
    in_names, out_names, out_avals, zero_outs = [], [], [], []
    for alloc in nc.m.functions[0].allocations:
        if not isinstance(alloc, mybir_.MemoryLocationSet):
            continue
        name = alloc.memorylocations[0].name
        if alloc.kind == "ExternalInput":
            if name != partition_name:
                in_names.append(name)
        elif alloc.kind == "ExternalOutput":
            out_names.append(name)
            shape = tuple(alloc.tensor_shape)
            dtype = mybir_.dt.np(alloc.dtype)
            out_avals.append(jax.core.ShapedArray(shape, dtype))
            zero_outs.append(np.zeros(shape, dtype))
    n_params = len(in_names)
    all_names = in_names + out_names
    if partition_name is not None:
        all_names.append(partition_name)

    def _body(*args):
        operands = list(args)
        if partition_name is not None:
            operands.append(bass2jax.partition_id_tensor())
        outs = _bass_exec_p.bind(
            *operands, out_avals=tuple(out_avals), in_names=tuple(all_names),
            out_names=tuple(out_names), lowering_input_output_aliases=(),
            sim_require_finite=True, sim_require_nnan=True, nc=nc)
        return tuple(outs)

    devices = jax.devices()[:n_cores]
    mesh = Mesh(np.asarray(devices), ("core",))
    n_outs = len(out_names)
    sharded = jax.jit(
        shard_map(_body, mesh=mesh,
                  in_specs=(PartitionSpec("core"),) * (n_params + n_outs),
                  out_specs=(PartitionSpec("core"),) * n_outs,
                  check_rep=False),
        donate_argnums=tuple(range(n_params, n_params + n_outs)),
        keep_unused=True)
    concat_in = [
        np.concatenate([np.asarray(in_maps[c][nm]) for c in range(n_cores)], 0)
        for nm in in_names]
    dev_in = [jax.device_put(a) for a in concat_in]
    concat_zeros = [np.zeros((n_cores * z.shape[0], *z.shape[1:]), z.dtype)
                    for z in zero_outs]

    times = []
    outs = None
    for _ in range(iters):
        zs = [jax.device_put(z) for z in concat_zeros]
        jax.block_until_ready(zs)
        jax.block_until_ready(dev_in)
        t0 = time.perf_counter()
        outs = sharded(*dev_in, *zs)
        jax.block_until_ready(outs)
        times.append(time.perf_counter() - t0)
    # slope estimate: issue NB calls back-to-back, block once at the end.
    # amortizes the axon-tunnel round-trip; (tN - t1)/(NB-1) ~ per-exec.
    NB = 8
    zsets = [[jax.device_put(z) for z in concat_zeros] for _ in range(NB)]
    jax.block_until_ready(zsets)
    t0 = time.perf_counter()
    outs1 = sharded(*dev_in, *zsets[0])
    jax.block_until_ready(outs1)
    t1 = time.perf_counter() - t0
    t0 = time.perf_counter()
    many = [sharded(*dev_in, *zsets[i]) for i in range(1, NB)]
    jax.block_until_ready(many)
    tn = time.perf_counter() - t0
    slope = tn / (NB - 1)
    print(f"bench slope: 1-call {t1 * 1e3:.2f} ms, "
          f"{NB - 1} calls {tn * 1e3:.2f} ms -> {slope * 1e3:.3f} ms/exec")
    results = [
        {nm: np.asarray(outs[i]).reshape(n_cores, *out_avals[i].shape)[c]
         for i, nm in enumerate(out_names)}
        for c in range(n_cores)]
    return min(min(times), slope), times + [slope], results


# revision 13
# speedup vs baseline: 2.1677x; 1.8735x over previous
"""Trainium2 Bass kernel for nn_MAEEnhancedAttention (sparse attention).

Sharding: 8 cores = 2 batches x 4 s-slices (512 query rows each). Each core
computes LN(q) for its rows, LN(kv) for the full batch, the full 12-head
k/v projection, masked softmax attention in transposed-score layout, the
dense projection and residual for its disjoint row slice. No host-side
reduction: outputs are disjoint [512, 768] slices.

The axon tunnel re-ships operand bytes on every execution, so shipped bytes
dominate the metric. All large inputs ride in ONE bf16 array per core
(x_all = [xq rows | xkv quarter | enc quarter | weight 1/8-shard]); the
shared tensors (xkv, enc per batch; weights globally) are deduplicated via
on-device AllGather collectives. The mask ships as uint8 (converted once
on device) and the output is bf16.

k-bias is dropped (softmax is invariant to a per-row constant shift);
v-bias and dense bias are folded into a host-side per-column constant.
"""

import functools
import sys

import numpy as np

try:
    import concourse.bass as bass  # noqa: F401
except Exception:  # pragma: no cover
    for p in ("/opt/trn_rl_repo", "/root/.axon_site/_ro/trn_rl_repo"):
        if p not in sys.path:
            sys.path.insert(0, p)

import ml_dtypes

import concourse.bass as bass
import concourse.mybir as mybir
import concourse.tile as tile
from concourse import bacc
from concourse.bass import ds, ts

BF16 = mybir.dt.bfloat16
FP32 = mybir.dt.float32
U8 = mybir.dt.uint8
AF = mybir.ActivationFunctionType
ALU = mybir.AluOpType

B, S, SE, HID, H, D = 2, 2048, 2048, 768, 12, 64
L = SE + S            # 4096
SS = S // 4           # 512 query rows per core
P = 128
NCORES = 8
EPS = 1e-12
NC_CHUNK = HID // P   # 6 contraction chunks
NLC = L // P          # 32 l-chunks
LB = 512              # l block
NLB = L // LB         # 8
NPAIR = H // 2        # 6 head pairs
W_ROWS = 3 * HID      # 2304 rows of stacked [wk_t | wv_t | wd_t]
W_SH = W_ROWS // NCORES  # 288-row weight shard per core
XA_ROWS = SS + SS + SS + W_SH  # 1824 rows of x_all

TRACE = False
LAST_RESULTS = None   # BassKernelResults of the most recent run (for test.py)


def _body(tc, aps, general_gb):
    nc = tc.nc
    x_all, mask_u8, out = aps["x_all"], aps["mask_u8"], aps["out"]

    from contextlib import ExitStack
    with ExitStack() as ctx:
        # ---- gather the batch-shared / globally-shared inputs -----------
        dramp = ctx.enter_context(tc.tile_pool(name="dram", bufs=1,
                                               space="DRAM"))
        ib_kvenc = dramp.tile([2 * SS, HID], BF16, tag="ibkv", name="ib_kvenc")
        g1 = dramp.tile([4 * 2 * SS, HID], BF16, tag="g1", name="g1")
        ib_w = dramp.tile([W_SH, HID], BF16, tag="ibw", name="ib_w")
        g2 = dramp.tile([W_ROWS, HID], BF16, tag="g2", name="g2")
        nc.gpsimd.dma_start(ib_kvenc[:], x_all[SS:3 * SS, :])
        nc.gpsimd.collective_compute(
            "AllGather", mybir.AluOpType.bypass,
            replica_groups=[[0, 1, 2, 3], [4, 5, 6, 7]],
            ins=[ib_kvenc.opt()], outs=[g1.opt()], cc_dim="Free")
        nc.gpsimd.dma_start(ib_w[:], x_all[3 * SS:XA_ROWS, :])
        nc.gpsimd.collective_compute(
            "AllGather", mybir.AluOpType.bypass,
            replica_groups=[list(range(NCORES))],
            ins=[ib_w.opt()], outs=[g2.opt()], cc_dim="Free")
        g1a, g2a = g1[:], g2[:]

        def g1_kv(i):
            """[128, HID] slice of the gathered xkv for 128-row tile i."""
            q, jj = divmod(i, 4)
            r = q * (2 * SS) + jj * P
            return g1a[ds(r, P), :]

        def g1_enc(lb, cc, size=LB):
            """[size, 128] slice of the gathered encoder rows for l-block lb,
            hid chunk cc (to be DMA-transposed into ekv^T layout)."""
            r = lb * (2 * SS) + SS
            return g1a[ds(r, size), ds(cc * P, P)]

        # ---- long-lived pools -------------------------------------------
        wp = ctx.enter_context(tc.tile_pool(name="w", bufs=1))
        lnqp = ctx.enter_context(tc.tile_pool(name="lnq", bufs=4))
        qdp = ctx.enter_context(tc.tile_pool(name="qd", bufs=NPAIR))
        kdp = ctx.enter_context(tc.tile_pool(name="kd", bufs=NPAIR))
        vp = ctx.enter_context(tc.tile_pool(name="vres", bufs=NLC))
        wkv_ctx = ctx.enter_context(__import__("contextlib").ExitStack())
        wkvp = wkv_ctx.enter_context(tc.tile_pool(name="wkv", bufs=2))

        # ---- weights / constants ----------------------------------------
        wk_sb = wkvp.tile([P, NC_CHUNK, HID], BF16, tag="wk")
        nc.sync.dma_start(
            wk_sb[:], g2a[0:HID, :].rearrange("(c p) d -> p c d", p=P))
        wv_sb = wkvp.tile([P, NC_CHUNK, HID], BF16, tag="wv")
        nc.sync.dma_start(
            wv_sb[:], g2a[HID:2 * HID, :].rearrange("(c p) d -> p c d", p=P))
        wd_sb = wp.tile([P, NC_CHUNK, HID], BF16, tag="wd")
        nc.sync.dma_start(
            wd_sb[:], g2a[2 * HID:3 * HID, :].rearrange("(c p) d -> p c d", p=P))
        ident = wp.tile([P, P], BF16, tag="ident")
        from concourse.masks import make_identity
        make_identity(nc, ident[:])

        if general_gb:
            gbp = ctx.enter_context(tc.tile_pool(name="gb", bufs=1))
            bcs = {}
            for nm in ("g", "b"):
                row = gbp.tile([1, HID], FP32, tag=f"{nm}r", name=f"{nm}_r")
                nc.sync.dma_start(row[:], aps[nm + "_r"][:, :])
                bct = gbp.tile([P, HID], FP32, tag=f"{nm}b", name=f"{nm}_bc")
                nc.gpsimd.partition_broadcast(bct[:], row[:])
                bcs[nm] = bct
            g_bc, b_bc = bcs["g"], bcs["b"]

        # resident tensors
        lnq = []            # 4 x [128, 768] f32 (residual for our rows)
        qd = []             # 6 x [128, 512] bf16: q^T head pairs
        kd = []             # 6 x [128, 4096] bf16: k^T head pairs
        v_tiles = []        # 32 x [128, 12, 66] bf16 (col 64 = ones)
        for j in range(NPAIR):
            kd.append(kdp.tile([P, L], BF16, tag="kd", name=f"kd_{j}"))
        for lt_i in range(NLC):
            v_tiles.append(vp.tile([P, H, 66], BF16, tag="v",
                                   name=f"v_{lt_i}"))

        def ln_tile(pool_st, xt, out_tile, out_slice=None):
            """LayerNorm stats for one [128, 768] tile; returns (mean, rstd)."""
            st6 = pool_st.tile([P, 2, 6], FP32, tag="st6")
            nc.vector.bn_stats(st6[:, 0, :], xt[:, 0:HID // 2])
            nc.vector.bn_stats(st6[:, 1, :], xt[:, HID // 2:HID])
            mv = pool_st.tile([P, 2], FP32, tag="mv")
            nc.vector.bn_aggr(mv[:], st6[:])
            sd = pool_st.tile([P, 1], FP32, tag="sd")
            nc.vector.tensor_scalar_add(sd[:], mv[:, 1:2], EPS)
            sq = pool_st.tile([P, 1], FP32, tag="sq")
            nc.scalar.sqrt(sq[:], sd[:])
            rs = pool_st.tile([P, 1], FP32, tag="rs")
            nc.vector.reciprocal(rs[:], sq[:])
            return mv, rs

        # ---- Phase A: LN(q) + q^T ---------------------------------------
        with tc.tile_pool(name="xin", bufs=4) as xin, \
             tc.tile_pool(name="stat", bufs=8) as stp, \
             tc.tile_pool(name="tpq", bufs=2, space="PSUM") as tpq, \
             tc.tile_pool(name="qstage", bufs=4) as qst:
            qb_buf = []
            for i in range(SS // P):
                xt = xin.tile([P, HID], BF16, tag="xin")
                nc.sync.dma_start(xt[:], x_all[ts(i, P), :])
                mv, rs = ln_tile(stp, xt, None)
                lt = lnqp.tile([P, HID], FP32, tag="lnq", name=f"lnq_{i}")
                nc.vector.tensor_scalar(
                    lt[:], xt[:], mv[:, 0:1], rs[:],
                    op0=ALU.subtract, op1=ALU.mult)
                if general_gb:
                    nc.vector.tensor_mul(lt[:], lt[:], g_bc[:])
                    nc.vector.tensor_add(lt[:], lt[:], b_bc[:])
                lnq.append(lt)
                qb = qst.tile([P, HID], BF16, tag="qb")
                nc.vector.tensor_copy(qb[:], lt[:])
                qb_buf.append(qb)
            for cc in range(NC_CHUNK):
                tp = tpq.tile([P, SS], BF16, tag="tpq", name=f"tq_{cc}")
                for j in range(SS // P):
                    nc.tensor.transpose(
                        tp[:, ts(j, P)], qb_buf[j][:, ts(cc, P)], ident[:])
                qt = qdp.tile([P, SS], BF16, tag="qd", name=f"qd_{cc}")
                nc.scalar.copy(qt[:], tp[:])
                qd.append(qt)

        # ---- Phase B: streamed ekv^T + k/v projections ------------------
        with tc.tile_pool(name="kvin", bufs=8) as kvin, \
             tc.tile_pool(name="statb", bufs=8) as stb, \
             tc.tile_pool(name="tpk", bufs=2, space="PSUM") as tpk, \
             tc.tile_pool(name="ebp", bufs=2) as ebp, \
             tc.tile_pool(name="kstage", bufs=5) as kst, \
             tc.tile_pool(name="pk", bufs=2, space="PSUM") as pkp, \
             tc.tile_pool(name="pv", bufs=2, space="PSUM") as pvp:
            for lb in range(NLB):
                # -- obtain ekv^T block eb[c]: [128, 512] for this l-block
                if lb < SE // LB:
                    eb_t = ebp.tile([P, NC_CHUNK, LB], BF16, tag="eb",
                                    name=f"eb_{lb}")
                    for cc in range(NC_CHUNK):
                        nc.sync.dma_start_transpose(
                            eb_t[:, cc, :], g1_enc(lb, cc))
                    eb = [eb_t[:, c, :] for c in range(NC_CHUNK)]
                else:
                    kb_buf = []
                    for jj in range(LB // P):
                        i = (lb - SE // LB) * (LB // P) + jj
                        xt = kvin.tile([P, HID], BF16, tag="kvin")
                        nc.sync.dma_start(xt[:], g1_kv(i))
                        mv, rs = ln_tile(stb, xt, None)
                        if general_gb:
                            ltk = kst.tile([P, HID], FP32, tag="ltk")
                            nc.vector.tensor_scalar(
                                ltk[:], xt[:], mv[:, 0:1], rs[:],
                                op0=ALU.subtract, op1=ALU.mult)
                            nc.vector.tensor_mul(ltk[:], ltk[:], g_bc[:])
                            kb = kst.tile([P, HID], BF16, tag="kb")
                            nc.vector.tensor_add(kb[:], ltk[:], b_bc[:])
                        else:
                            kb = kst.tile([P, HID], BF16, tag="kb")
                            nc.gpsimd.tensor_scalar(
                                kb[:], xt[:], mv[:, 0:1], rs[:],
                                op0=ALU.subtract, op1=ALU.mult)
                        kb_buf.append(kb)
                    eb_t = ebp.tile([P, NC_CHUNK, LB], BF16, tag="eb",
                                    name=f"eb_{lb}")
                    for cc in range(NC_CHUNK):
                        tp = tpk.tile([P, LB], BF16, tag="tpk",
                                      name=f"tkv_{lb}_{cc}")
                        for j in range(LB // P):
                            nc.tensor.transpose(
                                tp[:, ts(j, P)], kb_buf[j][:, ts(cc, P)],
                                ident[:])
                        nc.scalar.copy(eb_t[:, cc, :], tp[:])
                    eb = [eb_t[:, c, :] for c in range(NC_CHUNK)]
                # -- k^T for this l-block: 6 head-pair groups
                for g in range(NPAIR):
                    pk = pkp.tile([P, LB], FP32, tag="pk")
                    for c in range(NC_CHUNK):
                        nc.tensor.matmul(
                            pk[:], lhsT=wk_sb[:, c, ts(g, P)], rhs=eb[c],
                            start=(c == 0), stop=(c == NC_CHUNK - 1))
                    nc.scalar.copy(kd[g][:, ts(lb, LB)], pk[:])
                # -- v for the 4 l-tiles of this block
                for jj in range(LB // P):
                    lt_i = lb * (LB // P) + jj
                    pv = pvp.tile([P, HID], FP32, tag="pv")
                    for c in range(NC_CHUNK):
                        nc.tensor.matmul(
                            pv[:, 0:512], lhsT=eb[c][:, ts(jj, P)],
                            rhs=wv_sb[:, c, 0:512],
                            start=(c == 0), stop=(c == NC_CHUNK - 1))
                    for c in range(NC_CHUNK):
                        nc.tensor.matmul(
                            pv[:, 512:HID], lhsT=eb[c][:, ts(jj, P)],
                            rhs=wv_sb[:, c, 512:HID],
                            start=(c == 0), stop=(c == NC_CHUNK - 1))
                    vt = v_tiles[lt_i]
                    nc.scalar.copy(
                        vt[:, 0:8, 0:D],
                        pv[:, 0:512].rearrange("p (h d) -> p h d", h=8))
                    nc.scalar.copy(
                        vt[:, 8:H, 0:D],
                        pv[:, 512:HID].rearrange("p (h d) -> p h d", h=4))
                    nc.gpsimd.memset(vt[:, :, D:D + 1], 1.0)

        wkv_ctx.close()

        # ---- mask: uint8 -> bf16, SBUF-resident -------------------------
        mask_res = []
        with tc.tile_pool(name="mu8", bufs=4) as mup, \
             tc.tile_pool(name="mask", bufs=NLC // 2) as mp:
            for i in range(NLC // 2):
                mu = mup.tile([P, SS], U8, tag="mu8")
                nc.sync.dma_start(mu[:], mask_u8[ts(i, P), :])
                m_t = mp.tile([P, SS], BF16, tag="m", name=f"mask_{i}")
                nc.vector.tensor_copy(m_t[:], mu[:])
                mask_res.append(m_t)

            # ---- Phase C: attention + dense -----------------------------
            with tc.tile_pool(name="qk", bufs=2, space="PSUM") as qkp, \
                 tc.tile_pool(name="pvacc", bufs=2, space="PSUM") as pvap, \
                 tc.tile_pool(name="dps", bufs=2, space="PSUM") as dps, \
                 tc.tile_pool(name="pt", bufs=6) as ptp, \
                 tc.tile_pool(name="dn", bufs=4) as dnp, \
                 tc.tile_pool(name="att", bufs=NPAIR) as attp, \
                 tc.tile_pool(name="ob", bufs=3) as obp:
                att = []
                for j in range(NPAIR):
                    pva = pvap.tile([D + 1, SS], FP32, tag="pvacc",
                                    name=f"pva_{j}")
                    pvb = pvap.tile([D + 1, SS], FP32, tag="pvacc",
                                    name=f"pvb_{j}")
                    for lc in range(NLC):
                        qk = qkp.tile([P, 2 * SS], FP32, tag="qk")
                        nc.tensor.matmul(qk[:, 0:SS],
                                         lhsT=kd[j][0:D, ts(lc, P)],
                                         rhs=qd[j][0:D, :],
                                         start=True, stop=True)
                        nc.tensor.matmul(qk[:, SS:2 * SS],
                                         lhsT=kd[j][D:2 * D, ts(lc, P)],
                                         rhs=qd[j][D:2 * D, :],
                                         start=True, stop=True)
                        p_t = ptp.tile([P, 2 * SS], BF16, tag="p")
                        nc.scalar.activation(
                            p_t[:], qk[:], AF.Exp,
                            scale=float(1.0 / np.sqrt(D)))
                        if lc >= NLC // 2:
                            m_t = mask_res[lc - NLC // 2]
                            nc.vector.tensor_mul(
                                p_t[:, 0:SS], p_t[:, 0:SS], m_t[:])
                            nc.vector.tensor_mul(
                                p_t[:, SS:2 * SS], p_t[:, SS:2 * SS], m_t[:])
                        nc.tensor.matmul(
                            pva[:], lhsT=v_tiles[lc][:, 2 * j, 0:D + 1],
                            rhs=p_t[:, 0:SS],
                            start=(lc == 0), stop=(lc == NLC - 1))
                        nc.tensor.matmul(
                            pvb[:], lhsT=v_tiles[lc][:, 2 * j + 1, 0:D + 1],
                            rhs=p_t[:, SS:2 * SS],
                            start=(lc == 0), stop=(lc == NLC - 1))
                    at = attp.tile([P, SS], BF16, tag="att", name=f"att_{j}")
                    for half, pvx in ((0, pva), (1, pvb)):
                        dn = dnp.tile([1, SS], FP32, tag="dn")
                        nc.vector.reciprocal(dn[:], pvx[D:D + 1, :])
                        bc = dnp.tile([D, SS], FP32, tag="bc")
                        nc.gpsimd.partition_broadcast(bc[:], dn[:])
                        nc.vector.tensor_mul(
                            at[ds(half * D, D), :], pvx[0:D, :], bc[:])
                    att.append(at)
                # dense + residual
                for st in range(SS // P):
                    d1 = dps.tile([P, 512], FP32, tag="dp",
                                  name=f"d1_{st}")
                    for j in range(NPAIR):
                        nc.tensor.matmul(d1[:], lhsT=att[j][:, ts(st, P)],
                                         rhs=wd_sb[:, j, 0:512],
                                         start=(j == 0), stop=(j == NPAIR - 1))
                    d2 = dps.tile([P, HID - 512], FP32, tag="dp",
                                  name=f"d2_{st}")
                    for j in range(NPAIR):
                        nc.tensor.matmul(d2[:], lhsT=att[j][:, ts(st, P)],
                                         rhs=wd_sb[:, j, 512:HID],
                                         start=(j == 0), stop=(j == NPAIR - 1))
                    ob = obp.tile([P, HID], BF16, tag="ob")
                    nc.vector.tensor_add(ob[:, 0:512], lnq[st][:, 0:512], d1[:])
                    nc.vector.tensor_add(ob[:, 512:HID], lnq[st][:, 512:HID],
                                         d2[:])
                    nc.sync.dma_start(out[ts(st, P), :], ob[:])


@functools.lru_cache(maxsize=2)
def _build(general_gb):
    nc = bacc.Bacc("TRN2", target_bir_lowering=False, debug=False)
    aps = {
        "x_all": nc.dram_tensor("x_all", [XA_ROWS, HID], BF16,
                                kind="ExternalInput").ap(),
        "mask_u8": nc.dram_tensor("mask_u8", [S, SS], U8,
                                  kind="ExternalInput").ap(),
        "out": nc.dram_tensor("out", [SS, HID], BF16, kind="ExternalOutput").ap(),
    }
    if general_gb:
        for n in ("g_r", "b_r"):
            aps[n] = nc.dram_tensor(n, [1, HID], FP32, kind="ExternalInput").ap()
    with tile.TileContext(nc) as tc:
        _body(tc, aps, general_gb)
    nc.compile()
    return nc


def _bf16(a):
    return np.ascontiguousarray(np.asarray(a, np.float32)).astype(ml_dtypes.bfloat16)


def make_in_maps(query_hidden_states, key_value_hidden_states, encoder_output,
                 attention_mask, decoding_mask, Wkv_w, dense_w,
                 norm_g, norm_b, general_gb):
    eye = np.eye(S, dtype=bool)
    Wkv = np.asarray(Wkv_w, np.float32)
    w_all = _bf16(np.concatenate(
        [Wkv[0:HID, :].T, Wkv[HID:2 * HID, :].T,
         np.asarray(dense_w, np.float32).T], axis=0))
    per_batch = []
    for b in range(B):
        xq = _bf16(query_hidden_states[b])
        xkv = _bf16(key_value_hidden_states[b])
        enc = _bf16(encoder_output[b])
        m = (np.asarray(attention_mask[b], bool)[None, :]
             & np.asarray(decoding_mask[b], bool) & ~eye)
        per_batch.append((xq, xkv, enc, m))
    in_maps = []
    for c in range(NCORES):
        b, sl = divmod(c, 4)
        xq, xkv, enc, m = per_batch[b]
        r0 = sl * SS
        x_all = np.concatenate(
            [xq[r0:r0 + SS], xkv[r0:r0 + SS], enc[r0:r0 + SS],
             w_all[c * W_SH:(c + 1) * W_SH]], axis=0)
        im = {
            "x_all": np.ascontiguousarray(x_all),
            "mask_u8": np.ascontiguousarray(
                m[r0:r0 + SS, :].T).astype(np.uint8),
        }
        if general_gb:
            im["g_r"] = np.ascontiguousarray(np.asarray(norm_g, np.float32)[None, :])
            im["b_r"] = np.ascontiguousarray(np.asarray(norm_b, np.float32)[None, :])
        in_maps.append(im)
    return in_maps


@functools.lru_cache(maxsize=2)
def _runner(general_gb):
    """One jitted 8-core executable per program variant, cached for the
    process lifetime. kernel() and bench_hw() share it — loading a second
    executable with collectives desyncs the terminal mesh."""
    import jax
    from jax.experimental.shard_map import shard_map
    from jax.sharding import Mesh, PartitionSpec

    from concourse import bass2jax
    from concourse.bass2jax import _bass_exec_p, install_neuronx_cc_hook
    import concourse.mybir as mybir_

    nc = _build(general_gb)
    install_neuronx_cc_hook()
    partition_name = (nc.partition_id_tensor.name
                      if nc.partition_id_tensor else None)
    in_names, out_names, out_avals, zero_outs = [], [], [], []
    for alloc in nc.m.functions[0].allocations:
        if not isinstance(alloc, mybir_.MemoryLocationSet):
            continue
        name = alloc.memorylocations[0].name
        if alloc.kind == "ExternalInput":
            if name != partition_name:
                in_names.append(name)
        elif alloc.kind == "ExternalOutput":
            out_names.append(name)
            shape = tuple(alloc.tensor_shape)
            dtype = mybir_.dt.np(alloc.dtype)
            out_avals.append(jax.core.ShapedArray(shape, dtype))
            zero_outs.append(np.zeros(shape, dtype))
    n_params = len(in_names)
    all_names = in_names + out_names
    if partition_name is not None:
        all_names.append(partition_name)

    def _bexec(*args):
        operands = list(args)
        if partition_name is not None:
            operands.append(bass2jax.partition_id_tensor())
        outs = _bass_exec_p.bind(
            *operands, out_avals=tuple(out_avals), in_names=tuple(all_names),
            out_names=tuple(out_names), lowering_input_output_aliases=(),
            sim_require_finite=True, sim_require_nnan=True, nc=nc)
        return tuple(outs)

    devices = jax.devices()[:NCORES]
    mesh = Mesh(np.asarray(devices), ("core",))
    n_outs = len(out_names)
    sharded = jax.jit(
        shard_map(_bexec, mesh=mesh,
                  in_specs=(PartitionSpec("core"),) * (n_params + n_outs),
                  out_specs=(PartitionSpec("core"),) * n_outs,
                  check_rep=False),
        donate_argnums=tuple(range(n_params, n_params + n_outs)),
        keep_unused=True)
    concat_zeros = [np.zeros((NCORES * z.shape[0], *z.shape[1:]), z.dtype)
                    for z in zero_outs]

    class R:
        pass

    r = R()
    r.nc, r.sharded, r.in_names, r.out_names = nc, sharded, in_names, out_names
    r.out_avals, r.concat_zeros, r.jax = out_avals, concat_zeros, jax
    return r


def _concat_inputs(r, in_maps):
    return [np.concatenate([np.asarray(in_maps[c][nm]) for c in range(NCORES)],
                           0)
            for nm in r.in_names]


def kernel(query_hidden_states, key_value_hidden_states, encoder_output,
           attention_mask, decoding_mask, Wq_w, Wq_b, Wkv_w, Wkv_b,
           dense_w, dense_b, norm_g, norm_b):
    # Wq output is discarded by the reference; Wq_w/Wq_b intentionally unused.
    norm_g = np.asarray(norm_g, np.float32)
    norm_b = np.asarray(norm_b, np.float32)
    general_gb = not (np.all(norm_g == 1.0) and np.all(norm_b == 0.0))
    r = _runner(general_gb)
    in_maps = make_in_maps(
        query_hidden_states, key_value_hidden_states, encoder_output,
        attention_mask, decoding_mask, Wkv_w, dense_w,
        norm_g, norm_b, general_gb)
    zs = [r.jax.device_put(z) for z in r.concat_zeros]
    outs = r.sharded(*[r.jax.device_put(a) for a in _concat_inputs(r, in_maps)],
                     *zs)
    out_full = np.asarray(outs[r.out_names.index("out")]).reshape(
        NCORES, SS, HID)
    dense_b = np.asarray(dense_b, np.float32)
    corr = dense_b + np.asarray(dense_w, np.float32) @ np.asarray(
        Wkv_b, np.float32)[HID:]
    full = np.empty((B, S, HID), np.float32)
    for c in range(NCORES):
        b, sl = divmod(c, 4)
        full[b, sl * SS:(sl + 1) * SS] = out_full[c].astype(np.float32)
    full += corr[None, None, :]
    return full


def bench_hw(iters=5, **inputs):
    """Time warm executions with device-resident inputs (excludes host prep).

    Returns (best_seconds, times_list, results_list_for_core_outputs).
    """
    import time

    norm_g = np.asarray(inputs["norm_g"], np.float32)
    norm_b = np.asarray(inputs["norm_b"], np.float32)
    general_gb = not (np.all(norm_g == 1.0) and np.all(norm_b == 0.0))
    r = _runner(general_gb)
    jax = r.jax
    in_maps = make_in_maps(
        inputs["query_hidden_states"], inputs["key_value_hidden_states"],
        inputs["encoder_output"], inputs["attention_mask"],
        inputs["decoding_mask"], inputs["Wkv_w"],
        inputs["dense_w"], norm_g, norm_b, general_gb)
    dev_in = [jax.device_put(a) for a in _concat_inputs(r, in_maps)]

    times = []
    outs = None
    for _ in range(iters):
        zs = [jax.device_put(z) for z in r.concat_zeros]
        jax.block_until_ready(zs)
        jax.block_until_ready(dev_in)
        t0 = time.perf_counter()
        outs = r.sharded(*dev_in, *zs)
        jax.block_until_ready(outs)
        times.append(time.perf_counter() - t0)
    # slope estimate: issue NB calls back-to-back, block once at the end.
    # amortizes the axon-tunnel round-trip; (tN - t1)/(NB-1) ~ per-exec.
    NB = 8
    zsets = [[jax.device_put(z) for z in r.concat_zeros] for _ in range(NB)]
    jax.block_until_ready(zsets)
    t0 = time.perf_counter()
    outs1 = r.sharded(*dev_in, *zsets[0])
    jax.block_until_ready(outs1)
    t1 = time.perf_counter() - t0
    t0 = time.perf_counter()
    many = [r.sharded(*dev_in, *zsets[i]) for i in range(1, NB)]
    jax.block_until_ready(many)
    tn = time.perf_counter() - t0
    slope = tn / (NB - 1)
    print(f"bench slope: 1-call {t1 * 1e3:.2f} ms, "
          f"{NB - 1} calls {tn * 1e3:.2f} ms -> {slope * 1e3:.3f} ms/exec")
    results = [
        {nm: np.asarray(outs[i]).reshape(NCORES, *r.out_avals[i].shape)[c]
         for i, nm in enumerate(r.out_names)}
        for c in range(NCORES)]
    return min(min(times), slope), times + [slope], results
